# revision 1
# baseline (speedup 1.0000x reference)
"""Trainium2 Bass kernel for the 5x5 circular-padded conv
   y = conv5x5_circular(x[16,64,384,768], w[64,64,5,5]) + b.

Self-contained: shards the batch dim T=16 across 8 NeuronCores (2 images
per core), runs one SPMD Bass/Tile program, gathers the full output.

Per-core kernel: implicit GEMM over filter taps in float32r (1 cyc/row
on TensorE at even free dim >= 256, ~1.5e-4 rel err). fp32r forbids PE
column tiling (PSUM out must start at partition 0), so the 128-wide
array is filled via:
 - 2x row tiling: position T0 (SBUF partitions 0-63, x copy1) and T8
   (partitions 64-127, x copy2) stream rhs concurrently, each K=64.
 - M=128 output-shift packing: lhsT [64ci,128]: cols 0-63 = tap (dy,dx),
   cols 64-127 = tap (dy,dx+1); PSUM row 64+co, col n is a partial of
   output pixel n-1, merged with a +1 column shift (DVE cross-partition
   PSUM read). Taps (dy,4) zero the upper weight half (uniform 64x128
   tiling mode, no PE mode-switch drains).
Input is circularly padded on the host -> xp [2, 64, 388, 774].
"""

import numpy as np

import concourse.mybir as mybir
from concourse.tile import TileContext
from concourse import bacc
from concourse import bass_utils

F32 = mybir.dt.float32
F32R = mybir.dt.float32r
AFT = mybir.ActivationFunctionType

SLOT_DX0 = [0, 2, 4]
N_CORES = 8
T, C, H, W = 16, 64, 384, 768

_cache = {}


def _build_conv(T_loc, H, W, R=16):
    Hp, Wp = H + 4, W + 6
    Wh = W // 2
    Ns = Wh + 2
    nbands = H // R

    nc = bacc.Bacc("TRN2", target_bir_lowering=False, debug=False)
    xp = nc.dram_tensor("xp", [T_loc, C, Hp, Wp], F32R, kind="ExternalInput").ap()
    wd = nc.dram_tensor("wd", [128, 15 * 128], F32R, kind="ExternalInput").ap()
    bd = nc.dram_tensor("bd", [64, 1], F32, kind="ExternalInput").ap()
    y = nc.dram_tensor("y", [T_loc, C, H, W], F32, kind="ExternalOutput").ap()

    with TileContext(nc) as tc:
        with (
            tc.tile_pool(name="const", bufs=1) as cpool,
            tc.tile_pool(name="xband", bufs=2) as xpool,
            tc.tile_pool(name="yrow", bufs=6) as ypool,
            tc.tile_pool(name="psum", bufs=4, space="PSUM") as ppool,
        ):
            wsb = cpool.tile([128, 15 * 128], F32R)
            nc.sync.dma_start(out=wsb, in_=wd)
            bsb = cpool.tile([64, 1], F32)
            nc.sync.dma_start(out=bsb, in_=bd)

            for t in range(T_loc):
                for band in range(nbands):
                    r0 = band * R
                    xb = xpool.tile([128, R + 4, Wp], F32R)
                    nc.sync.dma_start(out=xb[0:64], in_=xp[t, :, r0 : r0 + R + 4, :])
                    nc.sync.dma_start(out=xb[64:128], in_=xb[0:64])
                    for h in range(R):
                        for wb in (0, Wh):
                            ps1 = ppool.tile([128, Ns], F32, tag="ps1")
                            ps2 = ppool.tile([128, Ns], F32, tag="ps2")
                            # Perfect T8/T0 alternation (incl. across tile
                            # boundaries) so the two row-group streams always
                            # overlap: T8 gets slots {14, 7..13}, T0 0..6.
                            seq = [(1, 14)]
                            for k in range(7):
                                seq.append((0, k))
                                seq.append((1, 7 + k))
                            for pos, s in seq:
                                dy, kk = divmod(s, 3)
                                dx0 = SLOT_DX0[kk]
                                lo, hi = (0, 64) if pos == 0 else (64, 128)
                                ps = ps1 if pos == 0 else ps2
                                nc.tensor.matmul(
                                    ps,
                                    wsb[lo:hi, s * 128 : (s + 1) * 128],
                                    xb[lo:hi, h + dy, wb + dx0 : wb + dx0 + Ns],
                                    start=(s == 0 or s == 14),
                                    stop=(s == 6 or s == 13),
                                )
                            t1 = ypool.tile([64, Wh], F32, tag="t1")
                            t2 = ypool.tile([64, Wh], F32, tag="t2")
                            nc.scalar.activation(t1, ps1[0:64, 0:Wh], AFT.Identity, bias=bsb)
                            nc.scalar.activation(t2, ps2[0:64, 0:Wh], AFT.Identity, bias=0.0)
                            nc.vector.tensor_add(out=t1, in0=t1, in1=ps1[64:128, 1 : Wh + 1])
                            nc.vector.tensor_add(out=t2, in0=t2, in1=ps2[64:128, 1 : Wh + 1])
                            nc.gpsimd.tensor_add(out=t1, in0=t1, in1=t2)
                            nc.sync.dma_start(out=y[t, :, r0 + h, wb : wb + Wh], in_=t1)
    nc.compile()
    return nc


def _make_wd(w):
    wd = np.zeros((64, 15, 128), dtype=np.float32)
    for dy in range(5):
        for k, dx0 in enumerate(SLOT_DX0):
            s = dy * 3 + k
            wd[:, s, 0:64] = w[:, :, dy, dx0].T
            if dx0 + 1 < 5:
                wd[:, s, 64:128] = w[:, :, dy, dx0 + 1].T
    wd = wd.reshape(64, 15 * 128)
    return np.ascontiguousarray(np.concatenate([wd, wd], axis=0))


def kernel(x, w, b):
    x = np.asarray(x, dtype=np.float32)
    w = np.asarray(w, dtype=np.float32)
    b = np.asarray(b, dtype=np.float32)
    assert x.shape == (T, C, H, W), x.shape

    T_loc = T // N_CORES
    if "nc" not in _cache:
        _cache["nc"] = _build_conv(T_loc, H, W)
    nc = _cache["nc"]

    xpad = np.pad(x, ((0, 0), (0, 0), (2, 2), (2, 4)), mode="wrap")
    wd = _make_wd(w)
    bd = b.reshape(64, 1).copy()
    in_maps = [
        {
            "xp": np.ascontiguousarray(xpad[c * T_loc : (c + 1) * T_loc]),
            "wd": wd,
            "bd": bd,
        }
        for c in range(N_CORES)
    ]
    res = bass_utils.run_bass_kernel_spmd(nc, in_maps, core_ids=list(range(N_CORES)))
    return np.concatenate([res.results[c]["y"] for c in range(N_CORES)], axis=0)



# revision 4
# speedup vs baseline: 1.4958x; 1.4958x over previous
"""Trainium2 Bass kernel for the 5x5 circular-padded conv
   y = conv5x5_circular(x[16,64,384,768], w[64,64,5,5]) + b.

Shards the batch dim T=16 across 8 NeuronCores (2 images per core),
runs one SPMD Bass/Tile program, gathers the full output.

Per-core kernel (v2, fp16 4-tile): direct conv as 25 taps of
K=64(ci), M=64(co) fp16 matmuls, keeping all four 64x64 PE quadrant
groups busy concurrently:
 - row split: image rows 0-191 live in SBUF partitions 0-63 (PE row
   group 0), rows 192-383 in partitions 64-127 (row group 64).
 - col split: even output rows accumulate in PSUM partitions 0-63
   (PE col group 0), odd rows in partitions 64-127 (col group 64).
Each PSUM bank hosts two output rows (even/odd partition halves,
independent accumulation chains). fp16 weights make the per-matmul
LDWEIGHTS cheap (64 cols ~53ns) vs fp32r (~206ns) which bounded v1.
Input is circularly padded and fp16-cast on the host -> [2,64,388,772].
"""

import numpy as np

import concourse.mybir as mybir
from concourse.tile import TileContext
from concourse import bacc
from concourse import bass_utils

F16 = mybir.dt.float16
F32 = mybir.dt.float32
AFT = mybir.ActivationFunctionType

N_CORES = 8
T, C, H, W = 16, 64, 384, 768
Hh = H // 2          # rows per PE row-group half
Hp, Wp = H + 4, W + 4
R = 4                # output rows per half per unit -> 8 banks in flight

_cache = {}


def _build_conv(T_loc):
    nunits = Hh // R
    nc = bacc.Bacc("TRN2", target_bir_lowering=False, debug=False)
    xp = nc.dram_tensor("xp", [T_loc, C, Hp, Wp], F16, kind="ExternalInput").ap()
    wd = nc.dram_tensor("wd", [128, 25 * 64], F16, kind="ExternalInput").ap()
    bd = nc.dram_tensor("bd", [128, 1], F32, kind="ExternalInput").ap()
    y = nc.dram_tensor("y", [T_loc, C, H, W], F32, kind="ExternalOutput").ap()

    with TileContext(nc) as tc:
        with (
            tc.tile_pool(name="const", bufs=1) as cpool,
            tc.tile_pool(name="xband", bufs=2) as xpool,
            tc.tile_pool(name="yrow", bufs=6) as ypool,
            tc.tile_pool(name="psum", bufs=1, space="PSUM") as ppool,
        ):
            wsb = cpool.tile([128, 25 * 64], F16)
            nc.sync.dma_start(out=wsb, in_=wd)
            bsb = cpool.tile([128, 1], F32)
            nc.sync.dma_start(out=bsb, in_=bd)

            for t in range(T_loc):
                for u in range(nunits):
                    r0 = u * R
                    xb = xpool.tile([128, R + 4, Wp], F16)
                    nc.sync.dma_start(out=xb[0:64], in_=xp[t, :, r0 : r0 + R + 4, :])
                    nc.sync.dma_start(
                        out=xb[64:128], in_=xp[t, :, Hh + r0 : Hh + r0 + R + 4, :]
                    )
                    ps = {}
                    for half in (0, 1):
                        for p in range(R // 2):
                            for wbi in (0, 1):
                                ps[(half, p, wbi)] = ppool.tile(
                                    [128, 384],
                                    F32,
                                    tag=f"ps{half}{p}{wbi}",
                                    name=f"ps{half}{p}{wbi}",
                                )
                    for s in range(25):
                        dy, dx = divmod(s, 5)
                        st = (s == 0)
                        sp = (s == 24)
                        for p in range(R // 2):
                            for wbi in (0, 1):
                                off = wbi * 384 + dx
                                for half in (0, 1):
                                    pb = 64 * half
                                    lhsT = wsb[pb : pb + 64, s * 64 : (s + 1) * 64]
                                    pst = ps[(half, p, wbi)]
                                    nc.tensor.matmul(
                                        pst[0:64],
                                        lhsT,
                                        xb[pb : pb + 64, 2 * p + dy, off : off + 384],
                                        start=st,
                                        stop=sp,
                                        skip_group_check=True,
                                    )
                                    nc.tensor.matmul(
                                        pst[64:128],
                                        lhsT,
                                        xb[pb : pb + 64, 2 * p + 1 + dy, off : off + 384],
                                        start=st,
                                        stop=sp,
                                        skip_group_check=True,
                                    )
                    for half in (0, 1):
                        for p in range(R // 2):
                            for wbi in (0, 1):
                                pst = ps[(half, p, wbi)]
                                tout = ypool.tile([128, 384], F32, tag="t")
                                nc.scalar.activation(tout, pst, AFT.Identity, bias=bsb)
                                h0 = half * Hh + r0 + 2 * p
                                wb = wbi * 384
                                nc.sync.dma_start(
                                    out=y[t, :, h0, wb : wb + 384], in_=tout[0:64]
                                )
                                nc.sync.dma_start(
                                    out=y[t, :, h0 + 1, wb : wb + 384], in_=tout[64:128]
                                )
    nc.compile()
    return nc


def prepare_in_maps(x, w, b):
    """Host-side prep: fp16 cast + circular pad, weight/bias layout, per-core shards."""
    x = np.asarray(x, dtype=np.float32)
    w = np.asarray(w, dtype=np.float32)
    b = np.asarray(b, dtype=np.float32)
    T_loc = T // N_CORES
    x16 = x.astype(np.float16)
    xpad = np.pad(x16, ((0, 0), (0, 0), (2, 2), (2, 2)), mode="wrap")
    # wd[ci, s*64+co] = w[co, ci, dy, dx], s = dy*5+dx; duplicated across
    # partition halves for the two PE row groups.
    wt = w.transpose(1, 2, 3, 0).reshape(64, 25 * 64).astype(np.float16)
    wdm = np.ascontiguousarray(np.concatenate([wt, wt], axis=0))
    bdm = np.concatenate([b, b]).reshape(128, 1).astype(np.float32)
    return [
        {
            "xp": np.ascontiguousarray(xpad[c * T_loc : (c + 1) * T_loc]),
            "wd": wdm,
            "bd": bdm,
        }
        for c in range(N_CORES)
    ]


def kernel(x, w, b):
    assert np.asarray(x).shape == (T, C, H, W)
    T_loc = T // N_CORES
    if "nc" not in _cache:
        _cache["nc"] = _build_conv(T_loc)
    nc = _cache["nc"]
    in_maps = prepare_in_maps(x, w, b)
    res = bass_utils.run_bass_kernel_spmd(nc, in_maps, core_ids=list(range(N_CORES)))
    return np.concatenate([res.results[c]["y"] for c in range(N_CORES)], axis=0)


# revision 5
# speedup vs baseline: 1.6463x; 1.1006x over previous
"""Trainium2 Bass kernel for the 5x5 circular-padded conv
   y = conv5x5_circular(x[16,64,384,768], w[64,64,5,5]) + b.

Shards the batch dim T=16 across 8 NeuronCores (2 images per core),
runs one SPMD Bass/Tile program, gathers the full output.

Per-core kernel (v3, fp16 4-tile + ping-pong PSUM): direct conv as 25
taps of K=64(ci), M=64(co) fp16 matmuls keeping all four 64x64 PE
quadrant groups busy concurrently:
 - row split: image rows 0-191 in SBUF partitions 0-63 (PE row group
   0), rows 192-383 in partitions 64-127 (row group 64).
 - col split: even output rows accumulate in PSUM partitions 0-63
   (col group 0), odd rows in partitions 64-127 (col group 64).
Steady-state per-tile cadence is the 384-col stream time (~163ns), so
the schedule only has to keep the PE fed: PSUM sub-units of 4 banks
(2 output rows per row-group half x 2 W-halves) ping-pong via bufs=2
pools while ScalarE+VectorE split the drains, so the PE never waits on
PSUM and the HAM clock stays at 2.4GHz (v2 lost ~0.5ms to drain stalls
and post-stall 1.2GHz restarts). Bands of 8 rows/half are double-
buffered and cover 4 sub-units each.
Input is circularly padded and fp16-cast on the host -> [2,64,388,772].
"""

import numpy as np

import concourse.mybir as mybir
from concourse.tile import TileContext
from concourse import bacc
from concourse import bass_utils

F16 = mybir.dt.float16
F32 = mybir.dt.float32
AFT = mybir.ActivationFunctionType

N_CORES = 8
T, C, H, W = 16, 64, 384, 768
Hh = H // 2          # rows per PE row-group half
Hp, Wp = H + 4, W + 4
RB = 8               # output rows per half per band
SU = 2               # output rows per half per PSUM sub-unit

_cache = {}


def _build_conv(T_loc):
    nbands = Hh // RB
    nsub = RB // SU
    nc = bacc.Bacc("TRN2", target_bir_lowering=False, debug=False)
    xp = nc.dram_tensor("xp", [T_loc, C, Hp, Wp], F16, kind="ExternalInput").ap()
    wd = nc.dram_tensor("wd", [128, 25 * 64], F16, kind="ExternalInput").ap()
    bd = nc.dram_tensor("bd", [128, 1], F32, kind="ExternalInput").ap()
    bdf = nc.dram_tensor("bdf", [128, 384], F32, kind="ExternalInput").ap()
    y = nc.dram_tensor("y", [T_loc, C, H, W], F32, kind="ExternalOutput").ap()

    with TileContext(nc) as tc:
        with (
            tc.tile_pool(name="const", bufs=1) as cpool,
            tc.tile_pool(name="xband", bufs=2) as xpool,
            tc.tile_pool(name="yrow", bufs=8) as ypool,
            tc.tile_pool(name="psum", bufs=2, space="PSUM") as ppool,
        ):
            wsb = cpool.tile([128, 25 * 64], F16)
            nc.sync.dma_start(out=wsb, in_=wd)
            bsb = cpool.tile([128, 1], F32)
            nc.sync.dma_start(out=bsb, in_=bd)
            bsf = cpool.tile([128, 384], F32)
            nc.sync.dma_start(out=bsf, in_=bdf)

            for t in range(T_loc):
                for u in range(nbands):
                    r0 = u * RB
                    xb = xpool.tile([128, RB + 4, Wp], F16)
                    nc.sync.dma_start(out=xb[0:64], in_=xp[t, :, r0 : r0 + RB + 4, :])
                    nc.sync.dma_start(
                        out=xb[64:128], in_=xp[t, :, Hh + r0 : Hh + r0 + RB + 4, :]
                    )
                    for j in range(nsub):
                        # sub-unit: rows r0+2j (even, col grp 0) and r0+2j+1
                        # (odd, col grp 64) for each row-group half.
                        ps = {}
                        for half in (0, 1):
                            for wbi in (0, 1):
                                ps[(half, wbi)] = ppool.tile(
                                    [128, 384],
                                    F32,
                                    tag=f"ps{half}{wbi}",
                                    name=f"ps{half}{wbi}",
                                )
                        for s in range(25):
                            dy, dx = divmod(s, 5)
                            st = (s == 0)
                            sp = (s == 24)
                            for wbi in (0, 1):
                                off = wbi * 384 + dx
                                for half in (0, 1):
                                    pb = 64 * half
                                    lhsT = wsb[pb : pb + 64, s * 64 : (s + 1) * 64]
                                    pst = ps[(half, wbi)]
                                    nc.tensor.matmul(
                                        pst[0:64],
                                        lhsT,
                                        xb[pb : pb + 64, 2 * j + dy, off : off + 384],
                                        start=st,
                                        stop=sp,
                                        skip_group_check=True,
                                    )
                                    nc.tensor.matmul(
                                        pst[64:128],
                                        lhsT,
                                        xb[pb : pb + 64, 2 * j + 1 + dy, off : off + 384],
                                        start=st,
                                        stop=sp,
                                        skip_group_check=True,
                                    )
                        for half in (0, 1):
                            for wbi in (0, 1):
                                pst = ps[(half, wbi)]
                                tout = ypool.tile([128, 384], F32, tag="t", name="t")
                                if half == 0:
                                    nc.scalar.activation(
                                        tout, pst, AFT.Identity, bias=bsb
                                    )
                                else:
                                    nc.vector.tensor_add(out=tout, in0=pst, in1=bsf)
                                h0 = half * Hh + r0 + 2 * j
                                wb = wbi * 384
                                nc.sync.dma_start(
                                    out=y[t, :, h0, wb : wb + 384], in_=tout[0:64]
                                )
                                nc.sync.dma_start(
                                    out=y[t, :, h0 + 1, wb : wb + 384], in_=tout[64:128]
                                )
    nc.compile()
    return nc


def prepare_in_maps(x, w, b):
    """Host-side prep: fp16 cast + circular pad, weight/bias layout, per-core shards."""
    x = np.asarray(x, dtype=np.float32)
    w = np.asarray(w, dtype=np.float32)
    b = np.asarray(b, dtype=np.float32)
    T_loc = T // N_CORES
    x16 = x.astype(np.float16)
    xpad = np.pad(x16, ((0, 0), (0, 0), (2, 2), (2, 2)), mode="wrap")
    # wd[ci, s*64+co] = w[co, ci, dy, dx], s = dy*5+dx; duplicated across
    # partition halves for the two PE row groups.
    wt = w.transpose(1, 2, 3, 0).reshape(64, 25 * 64).astype(np.float16)
    wdm = np.ascontiguousarray(np.concatenate([wt, wt], axis=0))
    b2 = np.concatenate([b, b]).astype(np.float32)
    bdm = b2.reshape(128, 1).copy()
    bdf = np.ascontiguousarray(np.broadcast_to(b2[:, None], (128, 384)))
    return [
        {
            "xp": np.ascontiguousarray(xpad[c * T_loc : (c + 1) * T_loc]),
            "wd": wdm,
            "bd": bdm,
            "bdf": bdf,
        }
        for c in range(N_CORES)
    ]


def kernel(x, w, b):
    assert np.asarray(x).shape == (T, C, H, W)
    T_loc = T // N_CORES
    if "nc" not in _cache:
        _cache["nc"] = _build_conv(T_loc)
    nc = _cache["nc"]
    in_maps = prepare_in_maps(x, w, b)
    res = bass_utils.run_bass_kernel_spmd(nc, in_maps, core_ids=list(range(N_CORES)))
    return np.concatenate([res.results[c]["y"] for c in range(N_CORES)], axis=0)


# revision 7
# speedup vs baseline: 1.9485x; 1.1836x over previous
"""Trainium2 Bass kernel for the 5x5 circular-padded conv
   y = conv5x5_circular(x[16,64,384,768], w[64,64,5,5]) + b.

Shards the batch dim T=16 across 8 NeuronCores (2 images per core),
runs one SPMD Bass/Tile program, gathers the full output.

Per-core kernel (v3, fp16 4-tile + ping-pong PSUM): direct conv as 25
taps of K=64(ci), M=64(co) fp16 matmuls keeping all four 64x64 PE
quadrant groups busy concurrently:
 - row split: image rows 0-191 in SBUF partitions 0-63 (PE row group
   0), rows 192-383 in partitions 64-127 (row group 64).
 - col split: even output rows accumulate in PSUM partitions 0-63
   (col group 0), odd rows in partitions 64-127 (col group 64).
Steady-state per-tile cadence is the 384-col stream time (~163ns), so
the schedule only has to keep the PE fed: PSUM sub-units of 4 banks
(2 output rows per row-group half x 2 W-halves) ping-pong via bufs=2
pools while ScalarE+VectorE split the drains, so the PE never waits on
PSUM and the HAM clock stays at 2.4GHz (v2 lost ~0.5ms to drain stalls
and post-stall 1.2GHz restarts). Bands of 8 rows/half are double-
buffered and cover 4 sub-units each.
Input is circularly padded and fp16-cast on the host -> [2,64,388,772].
"""

import numpy as np

import concourse.mybir as mybir
from concourse.tile import TileContext
from concourse import bacc
from concourse import bass_utils

F16 = mybir.dt.float16
F32 = mybir.dt.float32
AFT = mybir.ActivationFunctionType

N_CORES = 8
T, C, H, W = 16, 64, 384, 768
Hh = H // 2          # rows per PE row-group half
Hp, Wp = H + 4, W + 4
RB = 8               # output rows per half per band
SU = 2               # output rows per half per PSUM sub-unit

_cache = {}


def _build_conv(T_loc):
    nbands = Hh // RB
    nsub = RB // SU
    nc = bacc.Bacc("TRN2", target_bir_lowering=False, debug=False)
    xp = nc.dram_tensor("xp", [T_loc, C, Hp, Wp], F16, kind="ExternalInput").ap()
    wd = nc.dram_tensor("wd", [128, 25 * 64], F16, kind="ExternalInput").ap()
    bd = nc.dram_tensor("bd", [128, 1], F32, kind="ExternalInput").ap()
    bdf = nc.dram_tensor("bdf", [128, 384], F32, kind="ExternalInput").ap()
    y = nc.dram_tensor("y", [T_loc, C, H, W], F32, kind="ExternalOutput").ap()

    with TileContext(nc) as tc:
        with (
            tc.tile_pool(name="const", bufs=1) as cpool,
            tc.tile_pool(name="xband", bufs=3) as xpool,
            tc.tile_pool(name="yrow", bufs=8) as ypool,
            tc.tile_pool(name="psum", bufs=2, space="PSUM") as ppool,
        ):
            wsb = cpool.tile([128, 25 * 64], F16)
            nc.sync.dma_start(out=wsb, in_=wd)
            bsb = cpool.tile([128, 1], F32)
            nc.sync.dma_start(out=bsb, in_=bd)
            bsf = cpool.tile([128, 384], F32)
            nc.sync.dma_start(out=bsf, in_=bdf)

            # Input bands prefetched one band ahead on the sync HWDGE ring;
            # output DMAs ride the scalar-engine ring so a band load is never
            # stuck behind a band's worth of result stores in one FIFO.
            def load_band(t, u):
                r0 = u * RB
                xb = xpool.tile([128, RB + 4, Wp], F16, tag="xb", name="xb")
                nc.sync.dma_start(out=xb[0:64], in_=xp[t, :, r0 : r0 + RB + 4, :])
                nc.sync.dma_start(
                    out=xb[64:128], in_=xp[t, :, Hh + r0 : Hh + r0 + RB + 4, :]
                )
                return xb

            seq = [(t, u) for t in range(T_loc) for u in range(nbands)]
            xb_next = load_band(*seq[0])
            for idx, (t, u) in enumerate(seq):
                xb = xb_next
                if idx + 1 < len(seq):
                    xb_next = load_band(*seq[idx + 1])
                r0 = u * RB
                if True:
                    for j in range(nsub):
                        # sub-unit: rows r0+2j (even, col grp 0) and r0+2j+1
                        # (odd, col grp 64) for each row-group half.
                        ps = {}
                        for half in (0, 1):
                            for wbi in (0, 1):
                                ps[(half, wbi)] = ppool.tile(
                                    [128, 384],
                                    F32,
                                    tag=f"ps{half}{wbi}",
                                    name=f"ps{half}{wbi}",
                                )
                        for s in range(25):
                            dy, dx = divmod(s, 5)
                            st = (s == 0)
                            sp = (s == 24)
                            for wbi in (0, 1):
                                off = wbi * 384 + dx
                                for half in (0, 1):
                                    pb = 64 * half
                                    lhsT = wsb[pb : pb + 64, s * 64 : (s + 1) * 64]
                                    pst = ps[(half, wbi)]
                                    nc.tensor.matmul(
                                        pst[0:64],
                                        lhsT,
                                        xb[pb : pb + 64, 2 * j + dy, off : off + 384],
                                        start=st,
                                        stop=sp,
                                        skip_group_check=True,
                                    )
                                    nc.tensor.matmul(
                                        pst[64:128],
                                        lhsT,
                                        xb[pb : pb + 64, 2 * j + 1 + dy, off : off + 384],
                                        start=st,
                                        stop=sp,
                                        skip_group_check=True,
                                    )
                        for half in (0, 1):
                            for wbi in (0, 1):
                                pst = ps[(half, wbi)]
                                tout = ypool.tile([128, 384], F32, tag="t", name="t")
                                if half == 0:
                                    nc.scalar.activation(
                                        tout, pst, AFT.Identity, bias=bsb
                                    )
                                else:
                                    nc.vector.tensor_add(out=tout, in0=pst, in1=bsf)
                                h0 = half * Hh + r0 + 2 * j
                                wb = wbi * 384
                                nc.scalar.dma_start(
                                    out=y[t, :, h0, wb : wb + 384], in_=tout[0:64]
                                )
                                nc.scalar.dma_start(
                                    out=y[t, :, h0 + 1, wb : wb + 384], in_=tout[64:128]
                                )
    nc.compile()
    return nc


def prepare_in_maps(x, w, b):
    """Host-side prep: fp16 cast + circular pad, weight/bias layout, per-core shards."""
    x = np.asarray(x, dtype=np.float32)
    w = np.asarray(w, dtype=np.float32)
    b = np.asarray(b, dtype=np.float32)
    T_loc = T // N_CORES
    x16 = x.astype(np.float16)
    xpad = np.pad(x16, ((0, 0), (0, 0), (2, 2), (2, 2)), mode="wrap")
    # wd[ci, s*64+co] = w[co, ci, dy, dx], s = dy*5+dx; duplicated across
    # partition halves for the two PE row groups.
    wt = w.transpose(1, 2, 3, 0).reshape(64, 25 * 64).astype(np.float16)
    wdm = np.ascontiguousarray(np.concatenate([wt, wt], axis=0))
    b2 = np.concatenate([b, b]).astype(np.float32)
    bdm = b2.reshape(128, 1).copy()
    bdf = np.ascontiguousarray(np.broadcast_to(b2[:, None], (128, 384)))
    return [
        {
            "xp": np.ascontiguousarray(xpad[c * T_loc : (c + 1) * T_loc]),
            "wd": wdm,
            "bd": bdm,
            "bdf": bdf,
        }
        for c in range(N_CORES)
    ]


def kernel(x, w, b):
    assert np.asarray(x).shape == (T, C, H, W)
    T_loc = T // N_CORES
    if "nc" not in _cache:
        _cache["nc"] = _build_conv(T_loc)
    nc = _cache["nc"]
    in_maps = prepare_in_maps(x, w, b)
    res = bass_utils.run_bass_kernel_spmd(nc, in_maps, core_ids=list(range(N_CORES)))
    return np.concatenate([res.results[c]["y"] for c in range(N_CORES)], axis=0)
